# revision 1
# baseline (speedup 1.0000x reference)
"""DendriticFullyConnected Trainium2 kernel.

Math (per reference):
  x_c  = x[:, :409];  x_nc = x[:, 409:]
  state = sigmoid(x_nc @ W_non.T + b_non) - 1
  cluster = (x_c * coeff) @ W_nmda.T          # coeff = [1,2,...,2,1]
  pre = cluster + state
  out = pre^2 / (0.25 + pre^2)

Strategy: data-parallel over batch on 8 cores (1024 rows each), weights
replicated.  Host folds coeff into W_nmda, folds b_non in as an extra
contraction row (paired with a ones-row in x), transposes everything so the
contraction dim lands on SBUF partitions, and zero-pads K to multiples of 128:
   k-tiles 0..3   : nmda part (409 -> 512)
   k-tiles 4..32  : non part + bias row (3688 -> 3712)
Device computes outT[o, b] = sum_k wt[k, o] * xt[k, b] with W-stationary
matmuls (lhsT = wt tile [128k, 128o], rhs = xt [128k, 512b], float32r so the
PE runs at 1 cycle/row), two PSUM accumulation groups (nmda / non), then the
sigmoid + Hill epilogue on ACT/DVE.  Output is [O, B_loc]; host transposes
back and concatenates.

Scheduling: the x shard (16.5 MB) is cached in SBUF but its fill is
HBM-bound (~50 us) while the PE only has ~14 us of work per cached output
tile — a naive o-tile loop stalls ~30 us waiting for the tail of the fill.
So the first 4 o-tiles run k-OUTER: their nmda phases first (need only x
k-tiles 0..3), then all four non-phases advance one k-group at a time
(8 matmuls = ~1.8 us of PE work per arriving 1.5 us x tile), using all
8 PSUM banks.  Remaining o-tiles run the plain o-outer loop.

DMA layout: W streams through Sync/HWDGE in 4-k-tile chunks (the latency
critical feed for the PE); the one-time x cache fill and the output stores go
through GpSimd queues so they never head-of-line-block the W stream.
"""

import numpy as np

B = 8192
IN_F = 4096
OUT_F = 4096
IC = 409                      # clustering synapses
INC = IN_F - IC               # 3687
KD = 0.25                     # Hill k_d = k_a^n = 0.5^2
NCORES = 8
BLOC = B // NCORES            # 1024
KNM_PAD = 512                 # nmda contraction, padded
KNM_TILES = KNM_PAD // 128    # 4
KNON_PAD = 3712               # non contraction + bias row (3688), padded
KS = (KNM_PAD + KNON_PAD) // 128   # 33
KPAD = KS * 128               # 4224
BIAS_ROW = KNM_PAD + INC      # 4199: ones-row in x / b_non row in wt
OT = OUT_F // 128             # 32 output-row tiles
NBH = BLOC // 512             # 2 batch halves (512 = max fp32 matmul free dim)
OT_AHEAD = 4                  # o-tiles run k-outer to cover the x-cache fill

# Non-phase W-chunk schedule: groups of k-tiles fetched in one DMA each (7x4+1).
NON_GROUPS = [(k, min(4, KS - k)) for k in range(KNM_TILES, KS, 4)]

_nc_cache = []


def _build():
    import concourse.bacc as bacc
    import concourse.tile as tile
    import concourse.mybir as mybir

    f32 = mybir.dt.float32
    f32r = mybir.dt.float32r
    ACT = mybir.ActivationFunctionType

    nc = bacc.Bacc(None, target_bir_lowering=False)
    xt = nc.dram_tensor("xt", [KPAD, BLOC], f32, kind="ExternalInput")
    wt = nc.dram_tensor("wt", [KPAD, OUT_F], f32, kind="ExternalInput")
    outT = nc.dram_tensor("outT", [OUT_F, BLOC], f32, kind="ExternalOutput")

    with tile.TileContext(nc) as tc:
        with (
            tc.tile_pool(name="xpool", bufs=1) as xpool,
            tc.tile_pool(name="wpool", bufs=8) as wpool,
            tc.tile_pool(name="nmpool", bufs=12) as nmpool,
            tc.tile_pool(name="tmp", bufs=8) as tmp,
            tc.tile_pool(name="psum", bufs=8, space="PSUM") as psum,
        ):
            # Cache the full x shard in SBUF: 33 k-tiles of [128, 1024] f32r.
            # GpSimd queues: keeps the W stream on Sync unblocked.
            # Rows 409..511 and 4200..4223 of xT are structural zero padding:
            # memset them instead of spending fill-window HBM bandwidth.
            PAD = {3: 409 - 3 * 128, 32: BIAS_ROW + 1 - 32 * 128}  # real rows
            # The x fill is split across both DMA paths (even k-tiles on
            # Sync/HWDGE, odd on GpSimd/SWDGE) — a single path's queue set
            # caps well below what the HBM can deliver.  The Sync-side DMAs
            # are emitted lazily (interleaved with the W-chunk stream) via
            # feed_x() so they don't head-of-line-block W issues.
            xk = []
            x_pending = []
            for ks in range(KS):
                t = xpool.tile([128, BLOC], f32r, tag=f"x{ks}")
                rows = PAD.get(ks, 128)
                if rows < 128:
                    nc.vector.memset(t[:].bitcast(f32), 0.0)
                src = xt[ks * 128 : ks * 128 + rows, :].bitcast(f32r)
                if ks % 2 == 1 or ks < 4:
                    nc.gpsimd.dma_start(t[:rows, :], src)
                else:
                    x_pending.append((t, rows, src))
                xk.append(t)
            x_pending.reverse()  # pop() from the front of the schedule

            def feed_x(n):
                for _ in range(n):
                    if x_pending:
                        t, rows, src = x_pending.pop()
                        nc.sync.dma_start(t[:rows, :], src)

            def osl(ot):
                return slice(ot * 128, (ot + 1) * 128)

            def load_w_group(k0, g, ot):
                wg = wpool.tile([128, 4, 128], f32r, tag="w", name=f"w_{ot}_{k0}")
                src = wt[k0 * 128 : (k0 + g) * 128, osl(ot)].bitcast(f32r)
                nc.sync.dma_start(
                    wg[:, :g, :], src.rearrange("(g p) o -> p g o", p=128)
                )
                return wg

            def mm_sweep(psl, wg, k0, g, first_k, last_k):
                for j in range(g):
                    ks = k0 + j
                    for bh in range(NBH):
                        nc.tensor.matmul(
                            psl[bh][:],
                            lhsT=wg[:, j, :],
                            rhs=xk[ks][:, bh * 512 : (bh + 1) * 512],
                            start=(ks == first_k),
                            stop=(ks == last_k),
                        )

            def nmda_phase(ot):
                psn = [
                    psum.tile([128, 512], f32, tag="ps", name=f"psn_{ot}_{i}")
                    for i in range(NBH)
                ]
                wg = load_w_group(0, KNM_TILES, ot)
                feed_x(1)
                mm_sweep(psn, wg, 0, KNM_TILES, 0, KNM_TILES - 1)
                nm = []
                for bh in range(NBH):
                    t = nmpool.tile([128, 512], f32, tag="nm", name=f"nm_{ot}_{bh}")
                    nc.scalar.copy(t[:], psn[bh][:])
                    nm.append(t)
                return nm

            def epilogue_pair(ot, ps_pair, nm_pair):
                # pre = nm - sigmoid(-(z+b));  out = pre^2 / (KD + pre^2)
                # The two batch-half chains are interleaved so ACT and DVE
                # overlap instead of running one serial chain after the other.
                sig = [
                    tmp.tile([128, 512], f32, tag="t", name=f"sig_{ot}_{bh}")
                    for bh in range(NBH)
                ]
                rec = [
                    tmp.tile([128, 512], f32, tag="t", name=f"rec_{ot}_{bh}")
                    for bh in range(NBH)
                ]
                for bh in range(NBH):
                    nc.scalar.activation(sig[bh][:], ps_pair[bh][:], ACT.Sigmoid, scale=-1.0)
                for bh in range(NBH):
                    nc.vector.tensor_sub(sig[bh][:], nm_pair[bh][:], sig[bh][:])  # := pre
                for bh in range(NBH):
                    nc.scalar.activation(nm_pair[bh][:], sig[bh][:], ACT.Square)  # := pre^2
                for bh in range(NBH):
                    nc.vector.tensor_scalar_add(sig[bh][:], nm_pair[bh][:], KD)   # := den
                for bh in range(NBH):
                    nc.vector.reciprocal_approx_fast(rec[bh][:], sig[bh][:])
                for bh in range(NBH):
                    nc.vector.tensor_mul(nm_pair[bh][:], nm_pair[bh][:], rec[bh][:])
                for bh in range(NBH):
                    # ACT is the second HWDGE engine: stores ride its FIFO where
                    # they follow the epilogue anyway, never blocking the W
                    # stream on Sync and never paying the slow SWDGE tail drain.
                    bsl = slice(bh * 512, (bh + 1) * 512)
                    nc.scalar.dma_start(outT[osl(ot), bsl], nm_pair[bh][:])

            # ── Phase A: nmda for the first OT_AHEAD o-tiles (needs xk[0..3]) ──
            nm_ahead = [nmda_phase(ot) for ot in range(OT_AHEAD)]

            # ── Phase B: k-outer non-accumulation across those o-tiles ──
            ps_ahead = [
                [
                    psum.tile([128, 512], f32, tag="ps", name=f"psB_{ot}_{i}")
                    for i in range(NBH)
                ]
                for ot in range(OT_AHEAD)
            ]
            for k0, g in NON_GROUPS:
                wgs = [load_w_group(k0, g, ot) for ot in range(OT_AHEAD)]
                feed_x(2)
                # j-outer: each arriving x k-tile unlocks 2*OT_AHEAD matmuls,
                # keeping PE gaps below the HAM re-throttle window during the
                # x-cache fill.
                for j in range(g):
                    ks = k0 + j
                    for ot in range(OT_AHEAD):
                        for bh in range(NBH):
                            nc.tensor.matmul(
                                ps_ahead[ot][bh][:],
                                lhsT=wgs[ot][:, j, :],
                                rhs=xk[ks][:, bh * 512 : (bh + 1) * 512],
                                start=(ks == KNM_TILES),
                                stop=(ks == KS - 1),
                            )
            for ot in range(OT_AHEAD):
                epilogue_pair(ot, ps_ahead[ot], nm_ahead[ot])

            # ── Phase C: remaining o-tiles, plain o-outer loop ──
            for ot in range(OT_AHEAD, OT):
                nm = nmda_phase(ot)
                ps = [
                    psum.tile([128, 512], f32, tag="ps", name=f"ps_{ot}_{i}")
                    for i in range(NBH)
                ]
                for k0, g in NON_GROUPS:
                    wg = load_w_group(k0, g, ot)
                    feed_x(2)
                    mm_sweep(ps, wg, k0, g, KNM_TILES, KS - 1)
                epilogue_pair(ot, ps, nm)
    nc.compile()
    return nc


def _warmup():
    """Tiny throwaway NEFF run: the first execution after session start
    occasionally dies with NRT_EXEC_UNIT_UNRECOVERABLE; absorb that here."""
    import concourse.bacc as bacc
    import concourse.tile as tile
    import concourse.mybir as mybir
    from concourse.bass_utils import run_bass_kernel_spmd

    nc = bacc.Bacc(None, target_bir_lowering=False)
    a = nc.dram_tensor("a", [128, 128], mybir.dt.float32, kind="ExternalInput")
    b = nc.dram_tensor("b", [128, 128], mybir.dt.float32, kind="ExternalOutput")
    with tile.TileContext(nc) as tc:
        with tc.tile_pool(name="p", bufs=1) as pool:
            t = pool.tile([128, 128], mybir.dt.float32)
            nc.sync.dma_start(t[:], a[:])
            nc.sync.dma_start(b[:], t[:])
    nc.compile()
    ins = [{"a": np.zeros((128, 128), np.float32)} for _ in range(NCORES)]
    for _ in range(3):
        try:
            run_bass_kernel_spmd(nc, ins, core_ids=list(range(NCORES)))
            return
        except Exception:
            continue


def kernel(x, W_nmda, W_non, b_non):
    from concourse.bass_utils import run_bass_kernel_spmd

    x = np.asarray(x, dtype=np.float32)
    W_nmda = np.asarray(W_nmda, dtype=np.float32)
    W_non = np.asarray(W_non, dtype=np.float32)
    b_non = np.asarray(b_non, dtype=np.float32)

    coeff = np.full((IC,), 2.0, dtype=np.float32)
    coeff[0] = 1.0
    coeff[-1] = 1.0

    xT = np.zeros((KPAD, B), dtype=np.float32)
    xT[0:IC] = x[:, :IC].T
    xT[KNM_PAD : KNM_PAD + INC] = x[:, IC:].T
    xT[BIAS_ROW] = 1.0

    wt = np.zeros((KPAD, OUT_F), dtype=np.float32)
    wt[0:IC] = (W_nmda * coeff[None, :]).T
    wt[KNM_PAD : KNM_PAD + INC] = W_non.T
    wt[BIAS_ROW] = b_non

    in_maps = [
        {
            "xt": np.ascontiguousarray(xT[:, c * BLOC : (c + 1) * BLOC]),
            "wt": wt,
        }
        for c in range(NCORES)
    ]

    if not _nc_cache:
        _warmup()
        _nc_cache.append(_build())
    nc = _nc_cache[0]

    res = None
    last_exc = None
    for _attempt in range(3):
        try:
            res = run_bass_kernel_spmd(nc, in_maps, core_ids=list(range(NCORES)))
            break
        except Exception as e:  # transient device errors (e.g. first-run NRT hiccup)
            last_exc = e
    if res is None:
        raise last_exc

    global LAST_RESULT
    LAST_RESULT = res

    out = np.empty((B, OUT_F), dtype=np.float32)
    for c in range(NCORES):
        out[c * BLOC : (c + 1) * BLOC] = res.results[c]["outT"].T
    return out


LAST_RESULT = None



# revision 4
# speedup vs baseline: 1.7808x; 1.7808x over previous
"""DendriticFullyConnected Trainium2 kernel (fp8 DoubleRow version).

Math (per reference):
  x_c  = x[:, :409];  x_nc = x[:, 409:]
  state = sigmoid(x_nc @ W_non.T + b_non) - 1
  cluster = (x_c * coeff) @ W_nmda.T          # coeff = [1,2,...,2,1]
  pre = cluster + state
  out = pre^2 / (0.25 + pre^2)

Strategy: data-parallel over batch on 8 cores (1024 rows each), weights
replicated.  The big "non" contraction (3687 rows + 1 bias row, padded to
3840 = 15 pairs of 256) runs in fp8-e4m3 with perf_mode=DoubleRow: both
operands are quantized to e4m3 on the host (W_non scaled by 64 so its values
are O(1); the 1/64 is folded into the sigmoid's activation scale) and each
matmul contracts 256 rows at 2 fp8 MACs/cell/cycle.  The error lands before
the sigmoid, whose slope (<=0.25) attenuates it: measured rel-l2 ~5.4e-3
vs the 2e-2 budget.  The small "nmda" contraction (409 -> 512 rows, 12% of
FLOPs) feeds the Hill nonlinearity directly (slope up to ~1.3), so it stays
in float32r.  Output is stored as bf16 and upcast on the host.

Per-core layouts (host-prepared so every device DMA is contiguous):
  xnm [512, 1024] f32    nmda x, transposed       (one 2 MB DMA)
  xq  [1920, 2048] e4m3  non x, pair-interleaved: row g*128+p, col j*1024+b
                         = x_ncT[(2g+j)*128+p, b]  (15 x 256 KB DMAs)
  wnm [128, 32, 512] f32   wnm[p,ot,j*128+o]  = WmT[j*128+p,  ot*128+o]
  wq  [128, 32, 3840] e4m3 wq[p,ot,s*128+o]   = WqT[s*128+p, ot*128+o]
                         (one 2 KB/partition resp 3.75 KB/partition DMA
                          per o-tile)

Device loop: nmda phases for the first OT_AHEAD o-tiles run first (they only
need the small xnm cache) to cover the fp8 x-cache fill; then each o-tile
runs its 15 DoubleRow matmuls + 8 f32r matmuls per batch half, with the
sigmoid/Hill epilogue on ACT/DVE.  x fill is split across Sync (even pairs,
interleaved with the W stream via feed_x) and GpSimd (odd pairs) so neither
path head-of-line-blocks the W stream.
"""

import numpy as np

B = 8192
IN_F = 4096
OUT_F = 4096
IC = 409                       # clustering synapses
INC = IN_F - IC                # 3687
KD = 0.25                      # Hill k_d = k_a^n = 0.5^2
NCORES = 8
BLOC = B // NCORES             # 1024
KNM_PAD = 512                  # nmda contraction, padded
JNM = KNM_PAD // 128           # 4 nmda k-subtiles
NON_ROWS = INC + 1             # 3688: non rows + bias row
NPAIRS = 15                    # DoubleRow pairs of 256 rows
NON_PAD = NPAIRS * 256         # 3840
OT = OUT_F // 128              # 32 output-row tiles
NBH = BLOC // 512              # 2 batch halves (512 = max matmul out free dim)
WSCALE = 64.0                  # W_non prescale so e4m3 sees O(1) values
OT_AHEAD = 4                   # o-tiles whose nmda phase covers the x fill

_nc_cache = []


def _build():
    import concourse.bacc as bacc
    import concourse.tile as tile
    import concourse.mybir as mybir

    f32 = mybir.dt.float32
    f32r = mybir.dt.float32r
    fp8 = mybir.dt.float8e4
    bf16 = mybir.dt.bfloat16
    ACT = mybir.ActivationFunctionType
    DR = mybir.MatmulPerfMode.DoubleRow

    nc = bacc.Bacc(None, target_bir_lowering=False)
    xnm = nc.dram_tensor("xnm", [KNM_PAD, BLOC], f32, kind="ExternalInput")
    xq = nc.dram_tensor("xq", [NPAIRS * 128, 2 * BLOC], fp8, kind="ExternalInput")
    wnm = nc.dram_tensor("wnm", [128, OT, JNM * 128], f32, kind="ExternalInput")
    wq = nc.dram_tensor("wq", [128, OT, NPAIRS * 256], fp8, kind="ExternalInput")
    outT = nc.dram_tensor("outT", [OUT_F, BLOC], bf16, kind="ExternalOutput")

    with tile.TileContext(nc) as tc:
        with (
            tc.tile_pool(name="xpool", bufs=1) as xpool,
            tc.tile_pool(name="wqp", bufs=3) as wqp,
            tc.tile_pool(name="wnmp", bufs=3) as wnmp,
            tc.tile_pool(name="nmpool", bufs=10) as nmpool,
            tc.tile_pool(name="tmp", bufs=10) as tmp,
            tc.tile_pool(name="psum", bufs=8, space="PSUM") as psum,
        ):
            # ── x caches ────────────────────────────────────────────────
            # nmda x: [128, 4, 1024] f32r, first on the Sync queue (the
            # OT_AHEAD nmda phases need it before anything else).
            xnm_t = xpool.tile([128, JNM, BLOC], f32r, tag="xnm")
            nc.sync.dma_start(
                xnm_t[:],
                xnm[:, :].bitcast(f32r).rearrange("(j p) b -> p j b", p=128),
            )
            # non x: 15 pair tiles [128, 2, 1024] fp8.  Odd pairs fill on
            # GpSimd/SWDGE; even pairs ride Sync, issued after the Phase-A
            # wnm loads (below) but before the wq stream.  The whole non x
            # cache is only 3.75 MB, and non_sweep(0) — the first consumer —
            # doesn't start until the Phase-A nmda matmuls retire, so every
            # pair must be (and is) in flight before the first sweep.
            xq_t = []
            x_even = []
            for g in range(NPAIRS):
                t = xpool.tile([128, 2, BLOC], fp8, tag=f"xq{g}")
                src = xq[g * 128 : (g + 1) * 128, :].rearrange(
                    "p (j b) -> p j b", j=2
                )
                if g % 2 == 1:
                    nc.gpsimd.dma_start(t[:], src)
                else:
                    x_even.append((t, src))
                xq_t.append(t)

            def osl(ot):
                return slice(ot * 128, (ot + 1) * 128)

            def load_wnm(ot):
                t = wnmp.tile([128, JNM, 128], f32r, tag="wnm", name=f"wnm_{ot}")
                nc.sync.dma_start(
                    t[:],
                    wnm[:, ot, :].bitcast(f32r).rearrange("p (j o) -> p j o", j=JNM),
                )
                return t

            def load_wq(ot):
                t = wqp.tile([128, 2 * NPAIRS, 128], fp8, tag="wq", name=f"wq_{ot}")
                nc.sync.dma_start(
                    t[:],
                    wq[:, ot, :].rearrange("p (s o) -> p s o", s=2 * NPAIRS),
                )
                return t

            def nmda_phase(ot, wnm_t):
                ps = [
                    psum.tile([128, 512], f32, tag="ps", name=f"psnm_{ot}_{bh}")
                    for bh in range(NBH)
                ]
                for j in range(JNM):
                    for bh in range(NBH):
                        nc.tensor.matmul(
                            ps[bh][:],
                            lhsT=wnm_t[:, j, :],
                            rhs=xnm_t[:, j, bh * 512 : (bh + 1) * 512],
                            start=(j == 0),
                            stop=(j == JNM - 1),
                        )
                nm = []
                for bh in range(NBH):
                    t = nmpool.tile([128, 512], f32, tag="nm", name=f"nm_{ot}_{bh}")
                    nc.scalar.copy(t[:], ps[bh][:])
                    nm.append(t)
                return nm

            def non_sweep(ot, wq_t):
                ps = [
                    psum.tile([128, 512], f32, tag="ps", name=f"psno_{ot}_{bh}")
                    for bh in range(NBH)
                ]
                for g in range(NPAIRS):
                    for bh in range(NBH):
                        nc.tensor.matmul(
                            ps[bh][:],
                            lhsT=wq_t[:, 2 * g : 2 * g + 2, :],
                            rhs=xq_t[g][:, :, bh * 512 : (bh + 1) * 512],
                            start=(g == 0),
                            stop=(g == NPAIRS - 1),
                            perf_mode=DR,
                        )
                return ps

            def epilogue(ot, ps, nm):
                # pre = nm - sigmoid(-(z));  PSUM holds 64*z, so the sigmoid
                # scale is -1/64.  out = pre^2 / (KD + pre^2), stored bf16.
                # The two batch-half chains are interleaved so ACT and DVE
                # overlap instead of running one serial chain after the other.
                sig = [
                    tmp.tile([128, 512], f32, tag="t", name=f"sig_{ot}_{bh}")
                    for bh in range(NBH)
                ]
                rec = [
                    tmp.tile([128, 512], f32, tag="t", name=f"rec_{ot}_{bh}")
                    for bh in range(NBH)
                ]
                sq = [
                    tmp.tile([128, 512], f32, tag="t", name=f"sq_{ot}_{bh}")
                    for bh in range(NBH)
                ]
                ob = [
                    tmp.tile([128, 512], bf16, tag="ob", name=f"ob_{ot}_{bh}")
                    for bh in range(NBH)
                ]
                for bh in range(NBH):
                    nc.scalar.activation(
                        sig[bh][:], ps[bh][:], ACT.Sigmoid, scale=-1.0 / WSCALE
                    )
                for bh in range(NBH):
                    nc.vector.tensor_sub(sig[bh][:], nm[bh][:], sig[bh][:])  # := pre
                for bh in range(NBH):
                    nc.scalar.activation(sq[bh][:], sig[bh][:], ACT.Square)
                for bh in range(NBH):
                    nc.vector.tensor_scalar_add(sig[bh][:], sq[bh][:], KD)  # := den
                for bh in range(NBH):
                    nc.vector.reciprocal_approx_fast(rec[bh][:], sig[bh][:])
                for bh in range(NBH):
                    nc.vector.tensor_mul(ob[bh][:], sq[bh][:], rec[bh][:])
                for bh in range(NBH):
                    # ACT is the second HWDGE engine: stores ride its FIFO
                    # where they follow the epilogue anyway, never blocking
                    # the W stream on Sync.
                    bsl = slice(bh * 512, (bh + 1) * 512)
                    nc.scalar.dma_start(outT[osl(ot), bsl], ob[bh][:])

            # ── Phase A: nmda for the first OT_AHEAD o-tiles (covers the
            #    fp8 x-cache fill with PE work that only needs xnm) ──
            nm_done = {}
            for ot in range(OT_AHEAD):
                wnm_t = load_wnm(ot)
                # interleave the even-pair x fill with the Phase-A W loads
                for t, src in x_even[2 * ot : 2 * ot + 2]:
                    nc.sync.dma_start(t[:], src)
                nm_done[ot] = nmda_phase(ot, wnm_t)
            for t, src in x_even[2 * OT_AHEAD :]:
                nc.sync.dma_start(t[:], src)

            # ── Phase B: per-o-tile non sweep + pipelined nmda(ot+AHEAD) ──
            for ot in range(OT):
                wq_t = load_wq(ot)
                ps = non_sweep(ot, wq_t)
                if ot + OT_AHEAD < OT:
                    nm_done[ot + OT_AHEAD] = nmda_phase(
                        ot + OT_AHEAD, load_wnm(ot + OT_AHEAD)
                    )
                epilogue(ot, ps, nm_done.pop(ot))
    nc.compile()
    return nc


def _warmup():
    """Tiny throwaway NEFF run: the first execution after session start
    occasionally dies with NRT_EXEC_UNIT_UNRECOVERABLE; absorb that here."""
    import concourse.bacc as bacc
    import concourse.tile as tile
    import concourse.mybir as mybir
    from concourse.bass_utils import run_bass_kernel_spmd

    nc = bacc.Bacc(None, target_bir_lowering=False)
    a = nc.dram_tensor("a", [128, 128], mybir.dt.float32, kind="ExternalInput")
    b = nc.dram_tensor("b", [128, 128], mybir.dt.float32, kind="ExternalOutput")
    with tile.TileContext(nc) as tc:
        with tc.tile_pool(name="p", bufs=1) as pool:
            t = pool.tile([128, 128], mybir.dt.float32)
            nc.sync.dma_start(t[:], a[:])
            nc.sync.dma_start(b[:], t[:])
    nc.compile()
    ins = [{"a": np.zeros((128, 128), np.float32)} for _ in range(NCORES)]
    for _ in range(3):
        try:
            run_bass_kernel_spmd(nc, ins, core_ids=list(range(NCORES)))
            return
        except Exception:
            continue


def kernel(x, W_nmda, W_non, b_non):
    import ml_dtypes
    from concourse.bass_utils import run_bass_kernel_spmd

    e4 = ml_dtypes.float8_e4m3  # TRN fp8e4-compatible for |v| <= 240

    x = np.asarray(x, dtype=np.float32)
    W_nmda = np.asarray(W_nmda, dtype=np.float32)
    W_non = np.asarray(W_non, dtype=np.float32)
    b_non = np.asarray(b_non, dtype=np.float32)

    coeff = np.full((IC,), 2.0, dtype=np.float32)
    coeff[0] = 1.0
    coeff[-1] = 1.0

    # nmda weights (f32, coeff folded in): wnm[p, ot, j*128+o]
    Wm = np.zeros((KNM_PAD, OUT_F), dtype=np.float32)
    Wm[:IC] = (W_nmda * coeff[None, :]).T
    wnm_h = np.ascontiguousarray(
        Wm.reshape(JNM, 128, OT, 128).transpose(1, 2, 0, 3).reshape(128, OT, JNM * 128)
    )

    # non weights + bias row, scaled and quantized: wq[p, ot, s*128+o]
    Wq = np.zeros((NON_PAD, OUT_F), dtype=np.float32)
    Wq[:INC] = W_non.T * WSCALE
    Wq[INC] = b_non * WSCALE
    wq_h = np.ascontiguousarray(
        Wq.astype(e4)
        .reshape(NPAIRS, 2, 128, OT, 128)
        .transpose(2, 3, 0, 1, 4)
        .reshape(128, OT, NPAIRS * 256)
    )

    # x, transposed and split
    xnm_full = np.zeros((KNM_PAD, B), dtype=np.float32)
    xnm_full[:IC] = x[:, :IC].T
    Xn = np.zeros((NON_PAD, B), dtype=np.float32)
    Xn[:INC] = x[:, IC:].T
    Xn[INC] = 1.0  # bias row
    Xn8 = Xn.astype(e4)

    in_maps = []
    for c in range(NCORES):
        sl = slice(c * BLOC, (c + 1) * BLOC)
        xq_c = np.ascontiguousarray(
            Xn8[:, sl]
            .reshape(NPAIRS, 2, 128, BLOC)
            .transpose(0, 2, 1, 3)
            .reshape(NPAIRS * 128, 2 * BLOC)
        )
        in_maps.append(
            {
                "xnm": np.ascontiguousarray(xnm_full[:, sl]),
                "xq": xq_c,
                "wnm": wnm_h,
                "wq": wq_h,
            }
        )

    if not _nc_cache:
        _warmup()
        _nc_cache.append(_build())
    nc = _nc_cache[0]

    res = None
    last_exc = None
    for _attempt in range(3):
        try:
            res = run_bass_kernel_spmd(nc, in_maps, core_ids=list(range(NCORES)))
            break
        except Exception as e:  # transient device errors (e.g. first-run NRT hiccup)
            last_exc = e
    if res is None:
        raise last_exc

    global LAST_RESULT
    LAST_RESULT = res

    out = np.empty((B, OUT_F), dtype=np.float32)
    for c in range(NCORES):
        out[c * BLOC : (c + 1) * BLOC] = res.results[c]["outT"].astype(np.float32).T
    return out


LAST_RESULT = None


# revision 12
# speedup vs baseline: 1.8186x; 1.0212x over previous
"""DendriticFullyConnected Trainium2 kernel (fp8 DoubleRow version).

Math (per reference):
  x_c  = x[:, :409];  x_nc = x[:, 409:]
  state = sigmoid(x_nc @ W_non.T + b_non) - 1
  cluster = (x_c * coeff) @ W_nmda.T          # coeff = [1,2,...,2,1]
  pre = cluster + state
  out = pre^2 / (0.25 + pre^2)

Strategy: data-parallel over batch on 8 cores (1024 rows each), weights
replicated.  The big "non" contraction (3687 rows + 1 bias row, padded to
3840 = 15 pairs of 256) runs in fp8-e4m3 with perf_mode=DoubleRow: both
operands are quantized to e4m3 on the host (W_non scaled by 64 so its values
are O(1); the 1/64 is folded into the sigmoid's activation scale) and each
matmul contracts 256 rows at 2 fp8 MACs/cell/cycle.  The error lands before
the sigmoid, whose slope (<=0.25) attenuates it: measured rel-l2 ~5.4e-3
vs the 2e-2 budget.  The small "nmda" contraction (409 -> 512 rows, 12% of
FLOPs) feeds the Hill nonlinearity directly (slope up to ~1.3), so it stays
in float32r.  Output is stored as bf16 and upcast on the host.

Per-core layouts (host-prepared so every device DMA is contiguous):
  xnm [512, 1024] f32    nmda x, transposed       (one 2 MB DMA)
  xq  [1920, 2048] e4m3  non x, pair-interleaved: row g*128+p, col j*1024+b
                         = x_ncT[(2g+j)*128+p, b]  (15 x 256 KB DMAs)
  wnm [128, 32, 512] f32   wnm[p,ot,j*128+o]  = WmT[j*128+p,  ot*128+o]
  wq  [128, 32, 3840] e4m3 wq[p,ot,s*128+o]   = WqT[s*128+p, ot*128+o]
                         (one 2 KB/partition resp 3.75 KB/partition DMA
                          per o-tile)

Device loop: nmda phases for the first OT_AHEAD o-tiles run first (they only
need the small xnm cache) to cover the fp8 x-cache fill; then each o-tile
runs its 15 DoubleRow matmuls + 8 f32r matmuls per batch half, with the
sigmoid/Hill epilogue on ACT/DVE.  x fill is split across Sync (even pairs,
interleaved with the W stream via feed_x) and GpSimd (odd pairs) so neither
path head-of-line-blocks the W stream.
"""

import numpy as np

B = 8192
IN_F = 4096
OUT_F = 4096
IC = 409                       # clustering synapses
INC = IN_F - IC                # 3687
KD = 0.25                      # Hill k_d = k_a^n = 0.5^2
NCORES = 8
BLOC = B // NCORES             # 1024
KNM_PAD = 512                  # nmda contraction, padded
JNM = KNM_PAD // 128           # 4 nmda k-subtiles
NON_ROWS = INC + 1             # 3688: non rows + bias row
NPAIRS = 15                    # DoubleRow pairs of 256 rows
NON_PAD = NPAIRS * 256         # 3840
OT = OUT_F // 128              # 32 output-row tiles
NBH = BLOC // 512              # 2 batch halves (512 = max matmul out free dim)
WSCALE = 64.0                  # W_non prescale so e4m3 sees O(1) values
OT_AHEAD = 4                   # o-tiles whose nmda phase covers the x fill

_nc_cache = []


def _build():
    import concourse.bacc as bacc
    import concourse.tile as tile
    import concourse.mybir as mybir

    f32 = mybir.dt.float32
    f32r = mybir.dt.float32r
    fp8 = mybir.dt.float8e4
    bf16 = mybir.dt.bfloat16
    ACT = mybir.ActivationFunctionType
    DR = mybir.MatmulPerfMode.DoubleRow

    nc = bacc.Bacc(None, target_bir_lowering=False)
    xnm = nc.dram_tensor("xnm", [KNM_PAD, BLOC], bf16, kind="ExternalInput")
    xq = nc.dram_tensor("xq", [NPAIRS * 128, 2 * BLOC], fp8, kind="ExternalInput")
    wnm = nc.dram_tensor("wnm", [128, OT, JNM * 128], bf16, kind="ExternalInput")
    wq = nc.dram_tensor("wq", [128, OT, NPAIRS * 256], fp8, kind="ExternalInput")
    outT = nc.dram_tensor("outT", [OUT_F, BLOC], bf16, kind="ExternalOutput")

    with tile.TileContext(nc) as tc:
        with (
            tc.tile_pool(name="xpool", bufs=1) as xpool,
            tc.tile_pool(name="wqp", bufs=3) as wqp,
            tc.tile_pool(name="wnmp", bufs=3) as wnmp,
            tc.tile_pool(name="nmpool", bufs=10) as nmpool,
            tc.tile_pool(name="tmp", bufs=10) as tmp,
            tc.tile_pool(name="psum", bufs=8, space="PSUM") as psum,
        ):
            # ── x caches ────────────────────────────────────────────────
            # nmda x: 4 per-j tiles [128, 1024] bf16 so the first nmda
            # matmul only waits on j-tile 0 (~0.25 MB), not the whole cache.
            # They queue on Sync right after wnm_0 (issued in Phase A below).
            xnm_t = [
                xpool.tile([128, BLOC], bf16, tag=f"xnm{j}", name=f"xnm_{j}")
                for j in range(JNM)
            ]

            def fill_xnm():
                for j in range(JNM):
                    nc.sync.dma_start(xnm_t[j][:], xnm[j * 128 : (j + 1) * 128, :])
            # non x: 15 pair tiles [128, 2, 1024] fp8.  Odd pairs fill on
            # GpSimd/SWDGE; even pairs ride Sync, issued after the Phase-A
            # wnm loads (below) but before the wq stream.  The whole non x
            # cache is only 3.75 MB, and non_sweep(0) — the first consumer —
            # doesn't start until the Phase-A nmda matmuls retire, so every
            # pair must be (and is) in flight before the first sweep.
            xq_t = []
            x_even = []
            for g in range(NPAIRS):
                t = xpool.tile([128, 2, BLOC], fp8, tag=f"xq{g}")
                src = xq[g * 128 : (g + 1) * 128, :].rearrange(
                    "p (j b) -> p j b", j=2
                )
                if g % 2 == 1:
                    nc.gpsimd.dma_start(t[:], src)
                else:
                    x_even.append((t, src))
                xq_t.append(t)

            def osl(ot):
                return slice(ot * 128, (ot + 1) * 128)

            def load_wnm(ot):
                t = wnmp.tile([128, JNM, 128], bf16, tag="wnm", name=f"wnm_{ot}")
                nc.sync.dma_start(
                    t[:],
                    wnm[:, ot, :].rearrange("p (j o) -> p j o", j=JNM),
                )
                return t

            def load_wq(ot):
                t = wqp.tile([128, 2 * NPAIRS, 128], fp8, tag="wq", name=f"wq_{ot}")
                nc.sync.dma_start(
                    t[:],
                    wq[:, ot, :].rearrange("p (s o) -> p s o", s=2 * NPAIRS),
                )
                return t

            def nmda_phase(ot, wnm_t):
                ps = [
                    psum.tile([128, 512], f32, tag="ps", name=f"psnm_{ot}_{bh}")
                    for bh in range(NBH)
                ]
                for j in range(JNM):
                    for bh in range(NBH):
                        nc.tensor.matmul(
                            ps[bh][:],
                            lhsT=wnm_t[:, j, :],
                            rhs=xnm_t[j][:, bh * 512 : (bh + 1) * 512],
                            start=(j == 0),
                            stop=(j == JNM - 1),
                        )
                nm = []
                for bh in range(NBH):
                    t = nmpool.tile([128, 512], f32, tag="nm", name=f"nm_{ot}_{bh}")
                    nc.scalar.copy(t[:], ps[bh][:])
                    nm.append(t)
                return nm

            def non_sweep(ot, wq_t):
                ps = [
                    psum.tile([128, 512], f32, tag="ps", name=f"psno_{ot}_{bh}")
                    for bh in range(NBH)
                ]
                for g in range(NPAIRS):
                    for bh in range(NBH):
                        nc.tensor.matmul(
                            ps[bh][:],
                            lhsT=wq_t[:, 2 * g : 2 * g + 2, :],
                            rhs=xq_t[g][:, :, bh * 512 : (bh + 1) * 512],
                            start=(g == 0),
                            stop=(g == NPAIRS - 1),
                            perf_mode=DR,
                        )
                return ps

            def epilogue(ot, ps, nm):
                # pre = nm - sigmoid(-(z));  PSUM holds 64*z, so the sigmoid
                # scale is -1/64.  out = pre^2 / (KD + pre^2), stored bf16.
                # The two batch-half chains are interleaved so ACT and DVE
                # overlap instead of running one serial chain after the other.
                sig = [
                    tmp.tile([128, 512], f32, tag="t", name=f"sig_{ot}_{bh}")
                    for bh in range(NBH)
                ]
                rec = [
                    tmp.tile([128, 512], f32, tag="t", name=f"rec_{ot}_{bh}")
                    for bh in range(NBH)
                ]
                sq = [
                    tmp.tile([128, 512], f32, tag="t", name=f"sq_{ot}_{bh}")
                    for bh in range(NBH)
                ]
                ob = [
                    tmp.tile([128, 512], bf16, tag="ob", name=f"ob_{ot}_{bh}")
                    for bh in range(NBH)
                ]
                for bh in range(NBH):
                    nc.scalar.activation(
                        sig[bh][:], ps[bh][:], ACT.Sigmoid, scale=-1.0 / WSCALE
                    )
                for bh in range(NBH):
                    nc.vector.tensor_sub(sig[bh][:], nm[bh][:], sig[bh][:])  # := pre
                for bh in range(NBH):
                    nc.scalar.activation(sq[bh][:], sig[bh][:], ACT.Square)
                for bh in range(NBH):
                    nc.vector.tensor_scalar_add(sig[bh][:], sq[bh][:], KD)  # := den
                for bh in range(NBH):
                    nc.vector.reciprocal_approx_fast(rec[bh][:], sig[bh][:])
                for bh in range(NBH):
                    nc.vector.tensor_mul(ob[bh][:], sq[bh][:], rec[bh][:])
                for bh in range(NBH):
                    # ACT is the second HWDGE engine: stores ride its FIFO
                    # where they follow the epilogue anyway, never blocking
                    # the W stream on Sync.
                    bsl = slice(bh * 512, (bh + 1) * 512)
                    nc.scalar.dma_start(outT[osl(ot), bsl], ob[bh][:])

            # ── Phase A: nmda for the first OT_AHEAD o-tiles (covers the
            #    fp8 x-cache fill with PE work that only needs xnm) ──
            nm_done = {}
            for ot in range(OT_AHEAD):
                wnm_t = load_wnm(ot)
                if ot == 0:
                    fill_xnm()  # right behind wnm_0: first matmul waits
                    # only on wnm_0 + xnm j-tile 0 (~0.4 MB of DMA)
                # interleave the even-pair x fill with the Phase-A W loads
                for t, src in x_even[2 * ot : 2 * ot + 2]:
                    nc.sync.dma_start(t[:], src)
                nm_done[ot] = nmda_phase(ot, wnm_t)
            for t, src in x_even[2 * OT_AHEAD :]:
                nc.sync.dma_start(t[:], src)

            # ── Phase B: per-o-tile non sweep + pipelined nmda(ot+AHEAD) ──
            for ot in range(OT):
                wq_t = load_wq(ot)
                ps = non_sweep(ot, wq_t)
                if ot + OT_AHEAD < OT:
                    nm_done[ot + OT_AHEAD] = nmda_phase(
                        ot + OT_AHEAD, load_wnm(ot + OT_AHEAD)
                    )
                epilogue(ot, ps, nm_done.pop(ot))
    nc.compile()
    return nc


def _warmup():
    """Tiny throwaway NEFF run: the first execution after session start
    occasionally dies with NRT_EXEC_UNIT_UNRECOVERABLE; absorb that here."""
    import concourse.bacc as bacc
    import concourse.tile as tile
    import concourse.mybir as mybir
    from concourse.bass_utils import run_bass_kernel_spmd

    nc = bacc.Bacc(None, target_bir_lowering=False)
    a = nc.dram_tensor("a", [128, 128], mybir.dt.float32, kind="ExternalInput")
    b = nc.dram_tensor("b", [128, 128], mybir.dt.float32, kind="ExternalOutput")
    with tile.TileContext(nc) as tc:
        with tc.tile_pool(name="p", bufs=1) as pool:
            t = pool.tile([128, 128], mybir.dt.float32)
            nc.sync.dma_start(t[:], a[:])
            nc.sync.dma_start(b[:], t[:])
    nc.compile()
    ins = [{"a": np.zeros((128, 128), np.float32)} for _ in range(NCORES)]
    for _ in range(3):
        try:
            run_bass_kernel_spmd(nc, ins, core_ids=list(range(NCORES)))
            return
        except Exception:
            continue


def kernel(x, W_nmda, W_non, b_non):
    import ml_dtypes
    from concourse.bass_utils import run_bass_kernel_spmd

    e4 = ml_dtypes.float8_e4m3  # TRN fp8e4-compatible for |v| <= 240

    x = np.asarray(x, dtype=np.float32)
    W_nmda = np.asarray(W_nmda, dtype=np.float32)
    W_non = np.asarray(W_non, dtype=np.float32)
    b_non = np.asarray(b_non, dtype=np.float32)

    coeff = np.full((IC,), 2.0, dtype=np.float32)
    coeff[0] = 1.0
    coeff[-1] = 1.0

    # nmda weights (bf16, coeff folded in): wnm[p, ot, j*128+o]
    Wm = np.zeros((KNM_PAD, OUT_F), dtype=np.float32)
    Wm[:IC] = (W_nmda * coeff[None, :]).T
    wnm_h = np.ascontiguousarray(
        Wm.astype(ml_dtypes.bfloat16)
        .reshape(JNM, 128, OT, 128)
        .transpose(1, 2, 0, 3)
        .reshape(128, OT, JNM * 128)
    )

    # non weights + bias row, scaled and quantized: wq[p, ot, s*128+o]
    Wq = np.zeros((NON_PAD, OUT_F), dtype=np.float32)
    Wq[:INC] = W_non.T * WSCALE
    Wq[INC] = b_non * WSCALE
    wq_h = np.ascontiguousarray(
        Wq.astype(e4)
        .reshape(NPAIRS, 2, 128, OT, 128)
        .transpose(2, 3, 0, 1, 4)
        .reshape(128, OT, NPAIRS * 256)
    )

    # x, transposed and split
    xnm_full = np.zeros((KNM_PAD, B), dtype=ml_dtypes.bfloat16)
    xnm_full[:IC] = x[:, :IC].T.astype(ml_dtypes.bfloat16)
    Xn = np.zeros((NON_PAD, B), dtype=np.float32)
    Xn[:INC] = x[:, IC:].T
    Xn[INC] = 1.0  # bias row
    Xn8 = Xn.astype(e4)

    in_maps = []
    for c in range(NCORES):
        sl = slice(c * BLOC, (c + 1) * BLOC)
        xq_c = np.ascontiguousarray(
            Xn8[:, sl]
            .reshape(NPAIRS, 2, 128, BLOC)
            .transpose(0, 2, 1, 3)
            .reshape(NPAIRS * 128, 2 * BLOC)
        )
        in_maps.append(
            {
                "xnm": np.ascontiguousarray(xnm_full[:, sl]),
                "xq": xq_c,
                "wnm": wnm_h,
                "wq": wq_h,
            }
        )

    if not _nc_cache:
        _warmup()
        _nc_cache.append(_build())
    nc = _nc_cache[0]

    res = None
    last_exc = None
    for _attempt in range(3):
        try:
            res = run_bass_kernel_spmd(nc, in_maps, core_ids=list(range(NCORES)))
            break
        except Exception as e:  # transient device errors (e.g. first-run NRT hiccup)
            last_exc = e
    if res is None:
        raise last_exc

    global LAST_RESULT
    LAST_RESULT = res

    out = np.empty((B, OUT_F), dtype=np.float32)
    for c in range(NCORES):
        out[c * BLOC : (c + 1) * BLOC] = res.results[c]["outT"].astype(np.float32).T
    return out


LAST_RESULT = None
